# revision 18
# baseline (speedup 1.0000x reference)
"""GraphSAGE-style 2-layer GNN minibatch forward on 8 trn2 NeuronCores.

Data-parallel over the 1024 target nodes: each core handles 128 targets
(1408 layer-1 tokens). Host prep resolves the neighbor indices into a
per-core bf16 feature stream laid out transposed (feature dim on
partitions; per group: 128 self columns, then 25 slot-major [128-token]
neighbor slabs); the device kernel is a pure streaming pipeline: HWDGE
DMA loads each group's [128, 3328] half-tiles, the 25-neighbor sum runs
as a 6-op bf16 tensor-tensor ADD tree on DVE (fast-mode eligible,
unlike tensor_reduce), PE applies the MLP with the weight chunks as
stationary operands (no data transposes), and the l2-norm runs
per-group in the wide [128,1] orientation (Gram-column matmul, sqrt,
reciprocal, PE transpose to a row, rank-1 broadcast, DVE multiply) so
it overlaps the DMA-paced loop. All arithmetic (aggregation, matmuls,
relu, normalize) happens on device; the host only moves/permutes/casts
bytes.

All shapes hardcoded; self-contained (only needs the concourse runtime
that ships with the container).
"""

import numpy as np

N_CORES = 8
D = 256          # feature dim
P = 128          # partitions / tokens per group
B = 1024         # total targets
S0 = 25          # layer-0 fanout
S1 = 10          # layer-1 fanout
NG = 11          # groups of 128 tokens per core at layer 1 (1408 = 11*128)
NTOK = NG * P    # 1408 layer-1 tokens per core
NCOL = P * (1 + S0)   # 3328 columns per group tile: [self 128 | 25 slabs of 128]

_CACHE = {}


def _build_program():
    import concourse.bacc as bacc
    import concourse.mybir as mybir
    import concourse.tile as tile
    from concourse.masks import make_identity

    F32 = mybir.dt.float32
    BF16 = mybir.dt.bfloat16
    AF = mybir.ActivationFunctionType
    ALU = mybir.AluOpType
    AX = mybir.AxisListType

    nc = bacc.Bacc("TRN2", target_bir_lowering=False, debug=False)

    x0t_d = nc.dram_tensor("x0t", [NG * 2, P, NCOL], BF16, kind="ExternalInput")
    wc_d = nc.dram_tensor("wc", [P, 16 * P], BF16, kind="ExternalInput")
    bh_d = nc.dram_tensor("bh", [P, 4], F32, kind="ExternalInput")
    out_d = nc.dram_tensor("out", [P, D], F32, kind="ExternalOutput")

    with tile.TileContext(nc) as tc:
        with (
            tc.tile_pool(name="consts", bufs=1) as consts,
            tc.tile_pool(name="xp", bufs=4) as xp,
            tc.tile_pool(name="trp", bufs=2) as trp,
            tc.tile_pool(name="epp", bufs=2) as epp,
            tc.tile_pool(name="psh", bufs=3, space="PSUM") as psh,
            tc.tile_pool(name="psn", bufs=2, space="PSUM") as psn,
            tc.tile_pool(name="psr", bufs=2, space="PSUM") as psr,
            tc.tile_pool(name="psb", bufs=1, space="PSUM") as psb,
        ):
            # prefetch the first two groups before anything else hits the
            # sync DMA queue
            xtiles = {}

            def load_group(g):
                pair = []
                for h in range(2):
                    t = xp.tile([P, NCOL], BF16, tag=f"xh{h}", name=f"xh{h}_{g}")
                    nc.sync.dma_start(out=t[:], in_=x0t_d[g * 2 + h])
                    pair.append(t)
                xtiles[g] = pair

            load_group(0)
            load_group(1)

            w_all = consts.tile([P, 16 * P], BF16, tag="w_all")
            nc.sync.dma_start(out=w_all[:], in_=wc_d[:])
            b_all = consts.tile([P, 4], F32, tag="b_all")
            nc.sync.dma_start(out=b_all[:], in_=bh_d[:])
            w0_sb = [w_all[:, i * P:(i + 1) * P] for i in range(8)]
            w1_sb = [w_all[:, (8 + i) * P:(9 + i) * P] for i in range(8)]
            b0_sb = [b_all[:, h:h + 1] for h in range(2)]
            b1_sb = [b_all[:, 2 + h:3 + h] for h in range(2)]

            ident = consts.tile([P, P], F32, tag="ident")
            make_identity(nc, ident[:])
            ones1b = consts.tile([1, P], BF16, tag="ones1b")
            nc.vector.memset(ones1b[:], 1.0)
            ones1f = consts.tile([1, P], F32, tag="ones1f")
            nc.vector.memset(ones1f[:], 1.0)
            ones128b = consts.tile([P, 1], BF16, tag="ones128b")
            nc.vector.memset(ones128b[:], 1.0)
            ones128f = consts.tile([P, 1], F32, tag="ones128f")
            nc.vector.memset(ones128f[:], 1.0)
            epsb = consts.tile([1, 1], F32, tag="epsb")
            nc.vector.memset(epsb[:], 1e-30)
            epsp = consts.tile([P, 1], F32, tag="epsp")
            nc.vector.memset(epsp[:], 1e-30)
            # touch Sqrt early so its ACT table loads during startup
            warm = consts.tile([1, 1], F32, tag="warm")
            nc.scalar.activation(out=warm[:], in_=epsb[:], func=AF.Sqrt)

            h1_all = [
                consts.tile([P, NTOK], BF16, tag=f"h1_all{h}", name=f"h1_all{h}")
                for h in range(2)
            ]
            out_sb = consts.tile([P, D], F32, tag="out_sb")

            def agg_tree(xn, tag):
                # xn: [P, 25*P] bf16, 25 slot-major slabs; returns bf16 [P, P]
                # sum via tensor-tensor adds. The big first level runs on the
                # otherwise-idle gpsimd engine; the rest on DVE.
                t1 = trp.tile([P, 12 * P], BF16, tag=f"t1{tag}")
                nc.gpsimd.tensor_add(t1[:], xn[:, 0:12 * P], xn[:, 12 * P:24 * P])
                t2 = trp.tile([P, 6 * P], BF16, tag=f"t2{tag}")
                nc.vector.tensor_add(t2[:], t1[:, 0:6 * P], t1[:, 6 * P:12 * P])
                t3 = trp.tile([P, 3 * P], BF16, tag=f"t3{tag}")
                nc.vector.tensor_add(t3[:], t2[:, 0:3 * P], t2[:, 3 * P:6 * P])
                t4 = trp.tile([P, P], BF16, tag=f"t4{tag}")
                nc.vector.tensor_add(t4[:], t3[:, 0:P], t3[:, P:2 * P])
                t5 = trp.tile([P, P], BF16, tag=f"t5{tag}")
                nc.vector.tensor_add(t5[:], t4[:], t3[:, 2 * P:3 * P])
                ab = trp.tile([P, P], BF16, tag=f"ab{tag}")
                nc.vector.tensor_add(ab[:], t5[:], xn[:, 24 * P:25 * P])
                return ab

            def norm_scale(sqs, relu_src, dests, f32=False):
                # sqs: 2 sq half tiles; computes rinv per token and writes
                # dests[h] = relu_src[h] * rinv (column-wise scale).
                ones = ones128f if f32 else ones128b
                pn = psn.tile([P, 1], F32, tag="pn")
                nc.tensor.matmul(
                    out=pn[:], lhsT=sqs[0], rhs=ones[:], start=True, stop=False
                )
                nc.tensor.matmul(
                    out=pn[:], lhsT=sqs[1], rhs=ones[:], start=False, stop=True
                )
                ncol = epp.tile([P, 1], F32, tag="ncol")
                nc.scalar.activation(
                    out=ncol[:], in_=pn[:], func=AF.Sqrt, bias=epsp[:]
                )
                rcol = epp.tile([P, 1], F32, tag="rcol")
                nc.vector.reciprocal(out=rcol[:], in_=ncol[:])
                pr = psr.tile([1, P], F32, tag="pr")
                nc.tensor.matmul(
                    out=pr[:], lhsT=rcol[:], rhs=ident[:], start=True, stop=True
                )
                rrow = epp.tile([1, P], F32, tag="rrow")
                nc.scalar.copy(out=rrow[:], in_=pr[:])
                pb = psb.tile([P, P], F32, tag="pb")
                nc.tensor.matmul(
                    out=pb[:], lhsT=ones1f[:], rhs=rrow[:], start=True, stop=True
                )
                for h in range(2):
                    nc.vector.scalar_tensor_tensor(
                        out=dests[h], in0=relu_src[h], scalar=1.0, in1=pb[:],
                        op0=ALU.bypass, op1=ALU.mult,
                    )

            def mlp(x_chunks, w_sb, b_sb, hr_dests, edt):
                sqs = []
                for h in range(2):
                    ph = psh.tile([P, P], F32, tag="ph")
                    for k in range(4):
                        nc.tensor.matmul(
                            out=ph[:], lhsT=w_sb[k * 2 + h], rhs=x_chunks[k],
                            start=(k == 0), stop=(k == 3),
                        )
                    # bias is per-output-dim = per-partition here: ACT adds it
                    nc.scalar.activation(
                        out=hr_dests[h], in_=ph[:], func=AF.Relu, bias=b_sb[h]
                    )
                    sq = epp.tile([P, P], edt, tag=f"sq{h}")
                    nc.scalar.activation(out=sq[:], in_=hr_dests[h], func=AF.Square)
                    sqs.append(sq)
                return sqs

            # ---- layer 0: 11 groups of 128 tokens, streamed ----
            for g in range(NG):
                if g + 2 < NG:
                    load_group(g + 2)
                xh = xtiles.pop(g)
                aggb = [agg_tree(xh[h][:, P:], h) for h in range(2)]
                x_chunks = [xh[0][:, 0:P], xh[1][:, 0:P], aggb[0][:], aggb[1][:]]
                hr = [
                    epp.tile([P, P], BF16, tag=f"hr{h}", name=f"hr{h}_{g}")
                    for h in range(2)
                ]
                sqs = mlp(x_chunks, w0_sb, b0_sb, [hr[h][:] for h in range(2)], BF16)
                norm_scale(
                    [sqs[h][:] for h in range(2)],
                    [hr[h][:] for h in range(2)],
                    [h1_all[h][:, g * P:(g + 1) * P] for h in range(2)],
                )

            # ---- layer 1 ----
            agg1b = []
            for h in range(2):
                agg = trp.tile([P, P], F32, tag=f"agg1{h}")
                nc.vector.tensor_reduce(
                    out=agg[:],
                    in_=h1_all[h][:, P:].rearrange("p (t s) -> p t s", s=S1),
                    axis=AX.X, op=ALU.add,
                )
                ab = trp.tile([P, P], BF16, tag=f"agg1b{h}")
                nc.scalar.copy(out=ab[:], in_=agg[:])
                agg1b.append(ab)
            x_chunks = [
                h1_all[0][:, 0:P], h1_all[1][:, 0:P], agg1b[0][:], agg1b[1][:]
            ]
            hr2 = [
                epp.tile([P, P], F32, tag=f"hr2_{h}", name=f"hr2_{h}")
                for h in range(2)
            ]
            sqs2 = mlp(x_chunks, w1_sb, b1_sb, [hr2[h][:] for h in range(2)], F32)
            h2 = [
                epp.tile([P, P], F32, tag=f"h2_{h}", name=f"h2_{h}")
                for h in range(2)
            ]
            norm_scale(
                [sqs2[h][:] for h in range(2)],
                [hr2[h][:] for h in range(2)],
                [h2[h][:] for h in range(2)],
                f32=True,
            )
            # transpose h2T [d, t] -> out [t, d] and write
            for h in range(2):
                tp = psh.tile([P, P], F32, tag="ph")
                nc.tensor.transpose(out=tp[:], in_=h2[h][:], identity=ident[:])
                nc.scalar.copy(out=out_sb[:, h * P:(h + 1) * P], in_=tp[:])
            nc.sync.dma_start(out=out_d[:], in_=out_sb[:])

    nc.compile()
    return nc


def get_program():
    if "nc" not in _CACHE:
        _CACHE["nc"] = _build_program()
    return _CACHE["nc"]


def prepare_in_maps(features, W0, b0, W1, b1, nodes2, neigh2, neigh1):
    """Host-side sharding + index-resolved bf16 stream layout + weight prep."""
    import ml_dtypes

    bf16 = ml_dtypes.bfloat16
    feats16 = np.asarray(features, dtype=np.float32).astype(bf16)

    def chunk_w(W, fan):
        wt = np.ascontiguousarray(np.asarray(W, dtype=np.float32).T).copy()
        wt[D:, :] /= fan  # fold the neighbor mean into the weights
        # [k*2+h] = wt[k*128:(k+1)*128, h*128:(h+1)*128]
        return np.ascontiguousarray(
            wt.reshape(4, P, 2, P).transpose(0, 2, 1, 3).reshape(8, P, P)
        )

    w0c = chunk_w(W0, S0)
    w1c = chunk_w(W1, S1)
    # partition-major [P, 16P]: chunk i at columns [i*P, (i+1)*P)
    wc = np.concatenate([w0c, w1c], axis=0).transpose(1, 0, 2).reshape(P, 16 * P)
    wc = np.ascontiguousarray(wc).astype(bf16)
    # bias columns [P, 4]: (b0 lo, b0 hi, b1 lo, b1 hi), f32, per-partition
    bh = np.ascontiguousarray(
        np.concatenate(
            [np.asarray(b0, np.float32), np.asarray(b1, np.float32)]
        ).reshape(4, P).T
    )

    in_maps = []
    bc = B // N_CORES  # 128 targets per core
    for c in range(N_CORES):
        nodes2_c = nodes2[c * bc:(c + 1) * bc]
        neigh2_c = neigh2[c * bc:(c + 1) * bc, :]
        nodes1_c = np.concatenate([nodes2_c, neigh2_c.reshape(-1)])
        neigh1_c = np.concatenate(
            [
                neigh1[c * bc:(c + 1) * bc, :],
                neigh1[B + c * bc * S1:B + (c + 1) * bc * S1, :],
            ],
            axis=0,
        )
        # per group columns: [128 self | 25 slot-major slabs of 128 tokens]
        nodes1_g = nodes1_c.reshape(NG, P)
        neigh1_g = neigh1_c.reshape(NG, P, S0).transpose(0, 2, 1).reshape(NG, P * S0)
        flat = np.concatenate([nodes1_g, neigh1_g], axis=1).reshape(-1)
        rows = feats16[flat].reshape(NG, NCOL, D)
        x0t = np.ascontiguousarray(
            rows.transpose(0, 2, 1).reshape(NG, 2, P, NCOL)
        ).reshape(NG * 2, P, NCOL)
        in_maps.append({"x0t": x0t, "wc": wc, "bh": bh})
    return in_maps


def kernel(features, W0, b0, W1, b1, nodes2, neigh2, neigh1, _trace=False):
    from concourse.bass_utils import run_bass_kernel_spmd

    nc = get_program()
    in_maps = prepare_in_maps(features, W0, b0, W1, b1, nodes2, neigh2, neigh1)
    kwargs = {}
    if _trace:
        import tempfile

        import ntff_shim  # noqa: F401  (registers the axon NTFF hook)

        kwargs = {"trace": True, "tmpdir": tempfile.mkdtemp(prefix="ntff_")}
    res = run_bass_kernel_spmd(nc, in_maps, list(range(N_CORES)), **kwargs)
    out = np.concatenate([res.results[c]["out"] for c in range(N_CORES)], axis=0)
    if _trace:
        _CACHE["last_result"] = res
    return out


# revision 19
# speedup vs baseline: 1.4372x; 1.4372x over previous
"""GraphSAGE-style 2-layer GNN minibatch forward on 8 trn2 NeuronCores.

Data-parallel over the 1024 target nodes: each core handles 128 targets
(1408 layer-1 tokens). Host prep resolves the neighbor indices into a
per-core bf16 feature stream laid out transposed (feature dim on
partitions; per group and half: [128 self | slot-major slabs 0-12],
with slabs 13-24 in a second stream). The device kernel is a pure
streaming pipeline: an HWDGE DMA loads the first stream, then a SWDGE
accumulate-DMA adds the second stream onto the slab 0-11 region (the
first level of the 25-neighbor sum happens inside the DMA engines);
the remaining tree levels run as combined-half bf16 tensor-tensor adds
on DVE (2x fast mode), PE applies the MLP with the weight chunks as
stationary operands (no data transposes; bias is added by ACT as a
per-partition bias in the transposed layout), and the l2-norm runs
per-group in the wide [128,1] orientation (Gram-column matmul, sqrt,
reciprocal, PE transpose to a row, rank-1 broadcast, DVE multiply) so
it overlaps the DMA-paced loop. All arithmetic (aggregation, matmuls,
relu, normalize) happens on device; the host only moves/permutes/casts
bytes.

All shapes hardcoded; self-contained (only needs the concourse runtime
that ships with the container).
"""

import numpy as np

N_CORES = 8
D = 256          # feature dim
P = 128          # partitions / tokens per group
B = 1024         # total targets
S0 = 25          # layer-0 fanout
S1 = 10          # layer-1 fanout
NG = 11          # groups of 128 tokens per core at layer 1 (1408 = 11*128)
NTOK = NG * P    # 1408 layer-1 tokens per core
NA = P * 14      # stream A cols per half: self + slabs 0..12
NB = P * 12      # stream B cols per half: slabs 13..24

_CACHE = {}


def _build_program():
    import concourse.bacc as bacc
    import concourse.mybir as mybir
    import concourse.tile as tile
    from concourse.masks import make_identity

    F32 = mybir.dt.float32
    BF16 = mybir.dt.bfloat16
    AF = mybir.ActivationFunctionType
    ALU = mybir.AluOpType
    AX = mybir.AxisListType

    nc = bacc.Bacc("TRN2", target_bir_lowering=False, debug=False)

    x0a_d = nc.dram_tensor("x0a", [NG * 2, P, NA], BF16, kind="ExternalInput")
    x0b_d = nc.dram_tensor("x0b", [NG * 2, P, NB], BF16, kind="ExternalInput")
    wc_d = nc.dram_tensor("wc", [P, 16 * P], BF16, kind="ExternalInput")
    bh_d = nc.dram_tensor("bh", [P, 4], F32, kind="ExternalInput")
    out_d = nc.dram_tensor("out", [P, D], F32, kind="ExternalOutput")

    with tile.TileContext(nc) as tc:
        with (
            tc.tile_pool(name="consts", bufs=1) as consts,
            tc.tile_pool(name="xp", bufs=4) as xp,
            tc.tile_pool(name="trp", bufs=2) as trp,
            tc.tile_pool(name="epp", bufs=2) as epp,
            tc.tile_pool(name="psh", bufs=3, space="PSUM") as psh,
            tc.tile_pool(name="psn", bufs=2, space="PSUM") as psn,
            tc.tile_pool(name="psr", bufs=2, space="PSUM") as psr,
            tc.tile_pool(name="psb", bufs=1, space="PSUM") as psb,
        ):
            # prefetch the first groups before anything else hits the queues
            xtiles = {}

            def load_group(g):
                xt = xp.tile([P, 2 * NA], BF16, tag="xt", name=f"xt_{g}")
                for h in range(2):
                    nc.sync.dma_start(
                        out=xt[:, h * NA:(h + 1) * NA], in_=x0a_d[g * 2 + h]
                    )
                for h in range(2):
                    # DMA-fused first tree level: slabs 13-24 accumulate onto
                    # the slab 0-11 region (each dest address written once;
                    # ordered after the stream-A load by the tile tracker)
                    nc.gpsimd.dma_start(
                        out=xt[:, h * NA + P:h * NA + P + NB],
                        in_=x0b_d[g * 2 + h],
                        accum_op=ALU.add,
                    )
                xtiles[g] = xt

            load_group(0)
            load_group(1)

            w_all = consts.tile([P, 16 * P], BF16, tag="w_all")
            nc.sync.dma_start(out=w_all[:], in_=wc_d[:])
            b_all = consts.tile([P, 4], F32, tag="b_all")
            nc.sync.dma_start(out=b_all[:], in_=bh_d[:])
            w0_sb = [w_all[:, i * P:(i + 1) * P] for i in range(8)]
            w1_sb = [w_all[:, (8 + i) * P:(9 + i) * P] for i in range(8)]
            b0_sb = [b_all[:, h:h + 1] for h in range(2)]
            b1_sb = [b_all[:, 2 + h:3 + h] for h in range(2)]

            ident = consts.tile([P, P], F32, tag="ident")
            make_identity(nc, ident[:])
            ones1b = consts.tile([1, P], BF16, tag="ones1b")
            nc.vector.memset(ones1b[:], 1.0)
            ones1f = consts.tile([1, P], F32, tag="ones1f")
            nc.vector.memset(ones1f[:], 1.0)
            ones128b = consts.tile([P, 1], BF16, tag="ones128b")
            nc.vector.memset(ones128b[:], 1.0)
            ones128f = consts.tile([P, 1], F32, tag="ones128f")
            nc.vector.memset(ones128f[:], 1.0)
            epsb = consts.tile([1, 1], F32, tag="epsb")
            nc.vector.memset(epsb[:], 1e-30)
            epsp = consts.tile([P, 1], F32, tag="epsp")
            nc.vector.memset(epsp[:], 1e-30)
            # touch Sqrt early so its ACT table loads during startup
            warm = consts.tile([1, 1], F32, tag="warm")
            nc.scalar.activation(out=warm[:], in_=epsb[:], func=AF.Sqrt)

            # h1 (normalized layer-1 activations), both halves side by side
            h1_all = consts.tile(
                [P, 2 * NTOK], BF16, tag="h1_all", name="h1_all"
            )
            out_sb = consts.tile([P, D], F32, tag="out_sb")

            def agg_tree(xt, g):
                # After the fused DMA, per half: 12 paired slabs at cols
                # P..P+12*128 and raw slab 12 at cols P+12*128..NA.
                # Combined-half binary adds on DVE (all bf16 -> 2x mode).
                xv = xt[:].rearrange("p (a c) -> p a c", a=2)  # [p, 2, NA]
                t2 = trp.tile([P, 2 * 768], BF16, tag="t2")
                t2v = t2[:].rearrange("p (a c) -> p a c", a=2)
                nc.vector.tensor_add(
                    t2v, xv[:, :, P:P + 768], xv[:, :, P + 768:P + 1536]
                )
                t3 = trp.tile([P, 2 * 384], BF16, tag="t3")
                t3v = t3[:].rearrange("p (a c) -> p a c", a=2)
                nc.vector.tensor_add(t3v, t2v[:, :, 0:384], t2v[:, :, 384:768])
                t4 = trp.tile([P, 2 * P], BF16, tag="t4")
                t4v = t4[:].rearrange("p (a c) -> p a c", a=2)
                nc.vector.tensor_add(t4v, t3v[:, :, 0:P], t3v[:, :, P:2 * P])
                t5 = trp.tile([P, 2 * P], BF16, tag="t5")
                t5v = t5[:].rearrange("p (a c) -> p a c", a=2)
                nc.vector.tensor_add(
                    t5v, t3v[:, :, 2 * P:3 * P], xv[:, :, P + 1536:P + 1664]
                )
                ag = trp.tile([P, 2 * P], BF16, tag="ag")
                agv = ag[:].rearrange("p (a c) -> p a c", a=2)
                nc.vector.tensor_add(agv, t4v, t5v)
                return ag

            def norm_scale(sqs, relu_view, dest_view, f32=False):
                # sqs: 2 sq half tiles; relu_view/dest_view: [p, 2, P] APs.
                # Computes rinv per token, writes dest = relu * rinv.
                ones = ones128f if f32 else ones128b
                pn = psn.tile([P, 1], F32, tag="pn")
                nc.tensor.matmul(
                    out=pn[:], lhsT=sqs[0], rhs=ones[:], start=True, stop=False
                )
                nc.tensor.matmul(
                    out=pn[:], lhsT=sqs[1], rhs=ones[:], start=False, stop=True
                )
                ncol = epp.tile([P, 1], F32, tag="ncol")
                nc.scalar.activation(
                    out=ncol[:], in_=pn[:], func=AF.Sqrt, bias=epsp[:]
                )
                rcol = epp.tile([P, 1], F32, tag="rcol")
                nc.vector.reciprocal(out=rcol[:], in_=ncol[:])
                pr = psr.tile([1, P], F32, tag="pr")
                nc.tensor.matmul(
                    out=pr[:], lhsT=rcol[:], rhs=ident[:], start=True, stop=True
                )
                rrow = epp.tile([1, P], F32, tag="rrow")
                nc.scalar.copy(out=rrow[:], in_=pr[:])
                pb = psb.tile([P, P], F32, tag="pb")
                nc.tensor.matmul(
                    out=pb[:], lhsT=ones1f[:], rhs=rrow[:], start=True, stop=True
                )
                nc.vector.scalar_tensor_tensor(
                    out=dest_view, in0=relu_view, scalar=1.0,
                    in1=pb[:].unsqueeze(1).broadcast_to([P, 2, P]),
                    op0=ALU.bypass, op1=ALU.mult,
                )

            def mlp(x_chunks, w_sb, b_sb, hr_dests, edt):
                sqs = []
                for h in range(2):
                    ph = psh.tile([P, P], F32, tag="ph")
                    for k in range(4):
                        nc.tensor.matmul(
                            out=ph[:], lhsT=w_sb[k * 2 + h], rhs=x_chunks[k],
                            start=(k == 0), stop=(k == 3),
                        )
                    # bias is per-output-dim = per-partition here: ACT adds it
                    nc.scalar.activation(
                        out=hr_dests[h], in_=ph[:], func=AF.Relu, bias=b_sb[h]
                    )
                    sq = epp.tile([P, P], edt, tag=f"sq{h}")
                    nc.scalar.activation(out=sq[:], in_=hr_dests[h], func=AF.Square)
                    sqs.append(sq)
                return sqs

            # ---- layer 0: 11 groups of 128 tokens, streamed ----
            for g in range(NG):
                if g + 2 < NG:
                    load_group(g + 2)
                xt = xtiles.pop(g)
                ag = agg_tree(xt, g)
                x_chunks = [
                    xt[:, 0:P], xt[:, NA:NA + P], ag[:, 0:P], ag[:, P:2 * P]
                ]
                hr = epp.tile([P, 2 * P], BF16, tag="hr", name=f"hr_{g}")
                sqs = mlp(
                    x_chunks, w0_sb, b0_sb,
                    [hr[:, h * P:(h + 1) * P] for h in range(2)], BF16,
                )
                norm_scale(
                    [sqs[h][:] for h in range(2)],
                    hr[:].rearrange("p (a c) -> p a c", a=2),
                    h1_all[:].rearrange("p (a c) -> p a c", a=2)[
                        :, :, g * P:(g + 1) * P
                    ],
                )

            # ---- layer 1 ----
            agg1b = []
            for h in range(2):
                agg = trp.tile([P, P], F32, tag=f"agg1{h}")
                nc.vector.tensor_reduce(
                    out=agg[:],
                    in_=h1_all[:, h * NTOK + P:(h + 1) * NTOK].rearrange(
                        "p (t s) -> p t s", s=S1
                    ),
                    axis=AX.X, op=ALU.add,
                )
                ab = trp.tile([P, P], BF16, tag=f"agg1b{h}")
                nc.scalar.copy(out=ab[:], in_=agg[:])
                agg1b.append(ab)
            x_chunks = [
                h1_all[:, 0:P], h1_all[:, NTOK:NTOK + P],
                agg1b[0][:], agg1b[1][:],
            ]
            hr2 = epp.tile([P, 2 * P], F32, tag="hr2", name="hr2")
            sqs2 = mlp(
                x_chunks, w1_sb, b1_sb,
                [hr2[:, h * P:(h + 1) * P] for h in range(2)], F32,
            )
            h2 = epp.tile([P, 2 * P], F32, tag="h2", name="h2")
            norm_scale(
                [sqs2[h][:] for h in range(2)],
                hr2[:].rearrange("p (a c) -> p a c", a=2),
                h2[:].rearrange("p (a c) -> p a c", a=2),
                f32=True,
            )
            # transpose h2T [d, t] -> out [t, d] and write
            for h in range(2):
                tp = psh.tile([P, P], F32, tag="ph")
                nc.tensor.transpose(
                    out=tp[:], in_=h2[:, h * P:(h + 1) * P], identity=ident[:]
                )
                nc.scalar.copy(out=out_sb[:, h * P:(h + 1) * P], in_=tp[:])
            nc.sync.dma_start(out=out_d[:], in_=out_sb[:])

    nc.compile()
    return nc


def get_program():
    if "nc" not in _CACHE:
        _CACHE["nc"] = _build_program()
    return _CACHE["nc"]


def prepare_in_maps(features, W0, b0, W1, b1, nodes2, neigh2, neigh1):
    """Host-side sharding + index-resolved bf16 stream layout + weight prep."""
    import ml_dtypes

    bf16 = ml_dtypes.bfloat16
    feats16 = np.asarray(features, dtype=np.float32).astype(bf16)

    def chunk_w(W, fan):
        wt = np.ascontiguousarray(np.asarray(W, dtype=np.float32).T).copy()
        wt[D:, :] /= fan  # fold the neighbor mean into the weights
        # [k*2+h] = wt[k*128:(k+1)*128, h*128:(h+1)*128]
        return np.ascontiguousarray(
            wt.reshape(4, P, 2, P).transpose(0, 2, 1, 3).reshape(8, P, P)
        )

    w0c = chunk_w(W0, S0)
    w1c = chunk_w(W1, S1)
    # partition-major [P, 16P]: chunk i at columns [i*P, (i+1)*P)
    wc = np.concatenate([w0c, w1c], axis=0).transpose(1, 0, 2).reshape(P, 16 * P)
    wc = np.ascontiguousarray(wc).astype(bf16)
    # bias columns [P, 4]: (b0 lo, b0 hi, b1 lo, b1 hi), f32, per-partition
    bh = np.ascontiguousarray(
        np.concatenate(
            [np.asarray(b0, np.float32), np.asarray(b1, np.float32)]
        ).reshape(4, P).T
    )

    in_maps = []
    bc = B // N_CORES  # 128 targets per core
    for c in range(N_CORES):
        nodes2_c = nodes2[c * bc:(c + 1) * bc]
        neigh2_c = neigh2[c * bc:(c + 1) * bc, :]
        nodes1_c = np.concatenate([nodes2_c, neigh2_c.reshape(-1)])
        neigh1_c = np.concatenate(
            [
                neigh1[c * bc:(c + 1) * bc, :],
                neigh1[B + c * bc * S1:B + (c + 1) * bc * S1, :],
            ],
            axis=0,
        )
        # per group columns: [128 self | 25 slot-major slabs of 128 tokens]
        nodes1_g = nodes1_c.reshape(NG, P)
        neigh1_g = neigh1_c.reshape(NG, P, S0).transpose(0, 2, 1).reshape(NG, P * S0)
        flat = np.concatenate([nodes1_g, neigh1_g], axis=1).reshape(-1)
        rows = feats16[flat].reshape(NG, P * (1 + S0), D)
        # stream A: self + slabs 0..12; stream B: slabs 13..24
        xa = np.ascontiguousarray(
            rows[:, 0:NA, :].transpose(0, 2, 1).reshape(NG, 2, P, NA)
        ).reshape(NG * 2, P, NA)
        xb = np.ascontiguousarray(
            rows[:, NA:, :].transpose(0, 2, 1).reshape(NG, 2, P, NB)
        ).reshape(NG * 2, P, NB)
        in_maps.append({"x0a": xa, "x0b": xb, "wc": wc, "bh": bh})
    return in_maps


def kernel(features, W0, b0, W1, b1, nodes2, neigh2, neigh1, _trace=False):
    from concourse.bass_utils import run_bass_kernel_spmd

    nc = get_program()
    in_maps = prepare_in_maps(features, W0, b0, W1, b1, nodes2, neigh2, neigh1)
    kwargs = {}
    if _trace:
        import tempfile

        import ntff_shim  # noqa: F401  (registers the axon NTFF hook)

        kwargs = {"trace": True, "tmpdir": tempfile.mkdtemp(prefix="ntff_")}
    res = run_bass_kernel_spmd(nc, in_maps, list(range(N_CORES)), **kwargs)
    out = np.concatenate([res.results[c]["out"] for c in range(N_CORES)], axis=0)
    if _trace:
        _CACHE["last_result"] = res
    return out


# revision 24
# speedup vs baseline: 1.5964x; 1.1108x over previous
"""GraphSAGE-style 2-layer GNN minibatch forward on 8 trn2 NeuronCores.

Data-parallel over the 1024 target nodes: each core handles 128 targets
(1408 layer-1 tokens). Host prep resolves the neighbor indices into a
per-core bf16 feature stream laid out transposed (feature dim on
partitions; per group and half: [128 self | slot-major slabs 0-12],
with slabs 13-24 in a second stream). The device kernel is a pure
streaming pipeline: an HWDGE DMA loads the first stream, then a SWDGE
accumulate-DMA adds the second stream onto the slab 0-11 region (the
first level of the 25-neighbor sum happens inside the DMA engines);
the remaining tree levels run as combined-half bf16 tensor-tensor adds
on DVE (2x fast mode), PE applies the MLP with the weight chunks as
stationary operands (no data transposes; bias is added by ACT as a
per-partition bias in the transposed layout), and the l2-norm runs
per-group in the wide [128,1] orientation (Gram-column matmul, sqrt,
reciprocal, PE transpose to a row, rank-1 broadcast, DVE multiply) so
it overlaps the DMA-paced loop. All arithmetic (aggregation, matmuls,
relu, normalize) happens on device; the host only moves/permutes/casts
bytes.

All shapes hardcoded; self-contained (only needs the concourse runtime
that ships with the container).
"""

import numpy as np

N_CORES = 8
D = 256          # feature dim
P = 128          # partitions / tokens per group
B = 1024         # total targets
S0 = 25          # layer-0 fanout
S1 = 10          # layer-1 fanout
NG = 11          # groups of 128 tokens per core at layer 1 (1408 = 11*128)
NTOK = NG * P    # 1408 layer-1 tokens per core
NA = P * 26      # cols per half: self + 25 slot-major neighbor slabs

_CACHE = {}


def _build_program():
    import concourse.bacc as bacc
    import concourse.mybir as mybir
    import concourse.tile as tile
    from concourse.masks import make_identity

    F32 = mybir.dt.float32
    BF16 = mybir.dt.bfloat16
    AF = mybir.ActivationFunctionType
    ALU = mybir.AluOpType
    AX = mybir.AxisListType

    nc = bacc.Bacc("TRN2", target_bir_lowering=False, debug=False)

    x0a_d = nc.dram_tensor("x0a", [NG * 2, P, NA], BF16, kind="ExternalInput")
    wc_d = nc.dram_tensor("wc", [P, 16 * P], BF16, kind="ExternalInput")
    bh_d = nc.dram_tensor("bh", [P, 4], F32, kind="ExternalInput")
    out_d = nc.dram_tensor("out", [P, D], F32, kind="ExternalOutput")

    with tile.TileContext(nc) as tc:
        with (
            tc.tile_pool(name="consts", bufs=1) as consts,
            tc.tile_pool(name="xp", bufs=4) as xp,
            tc.tile_pool(name="trp", bufs=2) as trp,
            tc.tile_pool(name="epp", bufs=2) as epp,
            tc.tile_pool(name="psh", bufs=3, space="PSUM") as psh,
            tc.tile_pool(name="psn", bufs=2, space="PSUM") as psn,
            tc.tile_pool(name="psr", bufs=2, space="PSUM") as psr,
            tc.tile_pool(name="psb", bufs=1, space="PSUM") as psb,
        ):
            # prefetch the first groups before anything else hits the queues
            xtiles = {}

            def load_group(g):
                xt = xp.tile([P, 2 * NA], BF16, tag="xt", name=f"xt_{g}")
                for h in range(2):
                    nc.sync.dma_start(
                        out=xt[:, h * NA:(h + 1) * NA], in_=x0a_d[g * 2 + h]
                    )
                xtiles[g] = xt

            load_group(0)
            load_group(1)

            w_all = consts.tile([P, 16 * P], BF16, tag="w_all")
            nc.sync.dma_start(out=w_all[:], in_=wc_d[:])
            b_all = consts.tile([P, 4], F32, tag="b_all")
            nc.sync.dma_start(out=b_all[:], in_=bh_d[:])
            w0_sb = [w_all[:, i * P:(i + 1) * P] for i in range(8)]
            w1_sb = [w_all[:, (8 + i) * P:(9 + i) * P] for i in range(8)]
            b0_sb = [b_all[:, h:h + 1] for h in range(2)]
            b1_sb = [b_all[:, 2 + h:3 + h] for h in range(2)]

            ident = consts.tile([P, P], F32, tag="ident")
            make_identity(nc, ident[:])
            ones1b = consts.tile([1, P], BF16, tag="ones1b")
            nc.vector.memset(ones1b[:], 1.0)
            ones1f = consts.tile([1, P], F32, tag="ones1f")
            nc.vector.memset(ones1f[:], 1.0)
            ones128b = consts.tile([P, 1], BF16, tag="ones128b")
            nc.vector.memset(ones128b[:], 1.0)
            ones128f = consts.tile([P, 1], F32, tag="ones128f")
            nc.vector.memset(ones128f[:], 1.0)
            epsb = consts.tile([1, 1], F32, tag="epsb")
            nc.vector.memset(epsb[:], 1e-30)
            epsp = consts.tile([P, 1], F32, tag="epsp")
            nc.vector.memset(epsp[:], 1e-30)
            # touch Sqrt early so its ACT table loads during startup
            warm = consts.tile([1, 1], F32, tag="warm")
            nc.scalar.activation(out=warm[:], in_=epsb[:], func=AF.Sqrt)

            # h1 (normalized layer-1 activations), both halves side by side
            h1_all = consts.tile(
                [P, 2 * NTOK], BF16, tag="h1_all", name="h1_all"
            )
            out_sb = consts.tile([P, D], F32, tag="out_sb")

            def agg_tree(xt, g):
                # Per half: 25 slot-major slabs at cols P..NA. Combined-half
                # binary add tree on DVE (all bf16 -> 2x fast mode).
                xv = xt[:].rearrange("p (a c) -> p a c", a=2)  # [p, 2, NA]
                t1 = trp.tile([P, 2 * 1536], BF16, tag="t1")
                t1v = t1[:].rearrange("p (a c) -> p a c", a=2)
                nc.vector.tensor_add(
                    t1v, xv[:, :, P:P + 1536], xv[:, :, P + 1536:P + 3072]
                )
                t2 = trp.tile([P, 2 * 768], BF16, tag="t2")
                t2v = t2[:].rearrange("p (a c) -> p a c", a=2)
                nc.vector.tensor_add(t2v, t1v[:, :, 0:768], t1v[:, :, 768:1536])
                t3 = trp.tile([P, 2 * 384], BF16, tag="t3")
                t3v = t3[:].rearrange("p (a c) -> p a c", a=2)
                nc.vector.tensor_add(t3v, t2v[:, :, 0:384], t2v[:, :, 384:768])
                t4 = trp.tile([P, 2 * P], BF16, tag="t4")
                t4v = t4[:].rearrange("p (a c) -> p a c", a=2)
                nc.vector.tensor_add(t4v, t3v[:, :, 0:P], t3v[:, :, P:2 * P])
                t5 = trp.tile([P, 2 * P], BF16, tag="t5")
                t5v = t5[:].rearrange("p (a c) -> p a c", a=2)
                nc.vector.tensor_add(
                    t5v, t3v[:, :, 2 * P:3 * P], xv[:, :, P + 3072:P + 3200]
                )
                ag = trp.tile([P, 2 * P], BF16, tag="ag")
                agv = ag[:].rearrange("p (a c) -> p a c", a=2)
                nc.vector.tensor_add(agv, t4v, t5v)
                return ag

            def norm_scale(sqs, relu_view, dest_view, f32=False):
                # sqs: 2 sq half tiles; relu_view/dest_view: [p, 2, P] APs.
                # Computes rinv per token, writes dest = relu * rinv.
                ones = ones128f if f32 else ones128b
                pn = psn.tile([P, 1], F32, tag="pn")
                nc.tensor.matmul(
                    out=pn[:], lhsT=sqs[0], rhs=ones[:], start=True, stop=False
                )
                nc.tensor.matmul(
                    out=pn[:], lhsT=sqs[1], rhs=ones[:], start=False, stop=True
                )
                ncol = epp.tile([P, 1], F32, tag="ncol")
                nc.scalar.activation(
                    out=ncol[:], in_=pn[:], func=AF.Sqrt, bias=epsp[:]
                )
                rcol = epp.tile([P, 1], F32, tag="rcol")
                nc.vector.reciprocal(out=rcol[:], in_=ncol[:])
                pr = psr.tile([1, P], F32, tag="pr")
                nc.tensor.matmul(
                    out=pr[:], lhsT=rcol[:], rhs=ident[:], start=True, stop=True
                )
                rrow = epp.tile([1, P], F32, tag="rrow")
                nc.scalar.copy(out=rrow[:], in_=pr[:])
                pb = psb.tile([P, P], F32, tag="pb")
                nc.tensor.matmul(
                    out=pb[:], lhsT=ones1f[:], rhs=rrow[:], start=True, stop=True
                )
                nc.vector.scalar_tensor_tensor(
                    out=dest_view, in0=relu_view, scalar=1.0,
                    in1=pb[:].unsqueeze(1).broadcast_to([P, 2, P]),
                    op0=ALU.bypass, op1=ALU.mult,
                )

            def mlp(x_chunks, w_sb, b_sb, hr_dests, edt):
                sqs = []
                for h in range(2):
                    ph = psh.tile([P, P], F32, tag="ph")
                    for k in range(4):
                        nc.tensor.matmul(
                            out=ph[:], lhsT=w_sb[k * 2 + h], rhs=x_chunks[k],
                            start=(k == 0), stop=(k == 3),
                        )
                    # bias is per-output-dim = per-partition here: ACT adds it
                    nc.scalar.activation(
                        out=hr_dests[h], in_=ph[:], func=AF.Relu, bias=b_sb[h]
                    )
                    sq = epp.tile([P, P], edt, tag=f"sq{h}")
                    nc.scalar.activation(out=sq[:], in_=hr_dests[h], func=AF.Square)
                    sqs.append(sq)
                return sqs

            # ---- layer 0: 11 groups of 128 tokens, streamed ----
            for g in range(NG):
                if g + 2 < NG:
                    load_group(g + 2)
                xt = xtiles.pop(g)
                ag = agg_tree(xt, g)
                x_chunks = [
                    xt[:, 0:P], xt[:, NA:NA + P], ag[:, 0:P], ag[:, P:2 * P]
                ]
                hr = epp.tile([P, 2 * P], BF16, tag="hr", name=f"hr_{g}")
                sqs = mlp(
                    x_chunks, w0_sb, b0_sb,
                    [hr[:, h * P:(h + 1) * P] for h in range(2)], BF16,
                )
                norm_scale(
                    [sqs[h][:] for h in range(2)],
                    hr[:].rearrange("p (a c) -> p a c", a=2),
                    h1_all[:].rearrange("p (a c) -> p a c", a=2)[
                        :, :, g * P:(g + 1) * P
                    ],
                )

            # ---- layer 1 ----
            agg1b = []
            for h in range(2):
                agg = trp.tile([P, P], F32, tag=f"agg1{h}")
                nc.vector.tensor_reduce(
                    out=agg[:],
                    in_=h1_all[:, h * NTOK + P:(h + 1) * NTOK].rearrange(
                        "p (t s) -> p t s", s=S1
                    ),
                    axis=AX.X, op=ALU.add,
                )
                ab = trp.tile([P, P], BF16, tag=f"agg1b{h}")
                nc.scalar.copy(out=ab[:], in_=agg[:])
                agg1b.append(ab)
            x_chunks = [
                h1_all[:, 0:P], h1_all[:, NTOK:NTOK + P],
                agg1b[0][:], agg1b[1][:],
            ]
            hr2 = epp.tile([P, 2 * P], F32, tag="hr2", name="hr2")
            sqs2 = mlp(
                x_chunks, w1_sb, b1_sb,
                [hr2[:, h * P:(h + 1) * P] for h in range(2)], F32,
            )
            h2 = epp.tile([P, 2 * P], F32, tag="h2", name="h2")
            norm_scale(
                [sqs2[h][:] for h in range(2)],
                hr2[:].rearrange("p (a c) -> p a c", a=2),
                h2[:].rearrange("p (a c) -> p a c", a=2),
                f32=True,
            )
            # transpose h2T [d, t] -> out [t, d] and write
            for h in range(2):
                tp = psh.tile([P, P], F32, tag="ph")
                nc.tensor.transpose(
                    out=tp[:], in_=h2[:, h * P:(h + 1) * P], identity=ident[:]
                )
                nc.scalar.copy(out=out_sb[:, h * P:(h + 1) * P], in_=tp[:])
            nc.sync.dma_start(out=out_d[:], in_=out_sb[:])

    nc.compile()
    return nc


def get_program():
    if "nc" not in _CACHE:
        _CACHE["nc"] = _build_program()
    return _CACHE["nc"]


def prepare_in_maps(features, W0, b0, W1, b1, nodes2, neigh2, neigh1):
    """Host-side sharding + index-resolved bf16 stream layout + weight prep."""
    import ml_dtypes

    bf16 = ml_dtypes.bfloat16
    feats16 = np.asarray(features, dtype=np.float32).astype(bf16)

    def chunk_w(W, fan):
        wt = np.ascontiguousarray(np.asarray(W, dtype=np.float32).T).copy()
        wt[D:, :] /= fan  # fold the neighbor mean into the weights
        # [k*2+h] = wt[k*128:(k+1)*128, h*128:(h+1)*128]
        return np.ascontiguousarray(
            wt.reshape(4, P, 2, P).transpose(0, 2, 1, 3).reshape(8, P, P)
        )

    w0c = chunk_w(W0, S0)
    w1c = chunk_w(W1, S1)
    # partition-major [P, 16P]: chunk i at columns [i*P, (i+1)*P)
    wc = np.concatenate([w0c, w1c], axis=0).transpose(1, 0, 2).reshape(P, 16 * P)
    wc = np.ascontiguousarray(wc).astype(bf16)
    # bias columns [P, 4]: (b0 lo, b0 hi, b1 lo, b1 hi), f32, per-partition
    bh = np.ascontiguousarray(
        np.concatenate(
            [np.asarray(b0, np.float32), np.asarray(b1, np.float32)]
        ).reshape(4, P).T
    )

    in_maps = []
    bc = B // N_CORES  # 128 targets per core
    for c in range(N_CORES):
        nodes2_c = nodes2[c * bc:(c + 1) * bc]
        neigh2_c = neigh2[c * bc:(c + 1) * bc, :]
        nodes1_c = np.concatenate([nodes2_c, neigh2_c.reshape(-1)])
        neigh1_c = np.concatenate(
            [
                neigh1[c * bc:(c + 1) * bc, :],
                neigh1[B + c * bc * S1:B + (c + 1) * bc * S1, :],
            ],
            axis=0,
        )
        # per group columns: [128 self | 25 slot-major slabs of 128 tokens]
        nodes1_g = nodes1_c.reshape(NG, P)
        neigh1_g = neigh1_c.reshape(NG, P, S0).transpose(0, 2, 1).reshape(NG, P * S0)
        flat = np.concatenate([nodes1_g, neigh1_g], axis=1).reshape(-1)
        rows = feats16[flat].reshape(NG, P * (1 + S0), D)
        xa = np.ascontiguousarray(
            rows.transpose(0, 2, 1).reshape(NG, 2, P, NA)
        ).reshape(NG * 2, P, NA)
        in_maps.append({"x0a": xa, "wc": wc, "bh": bh})
    return in_maps


def kernel(features, W0, b0, W1, b1, nodes2, neigh2, neigh1, _trace=False):
    from concourse.bass_utils import run_bass_kernel_spmd

    nc = get_program()
    in_maps = prepare_in_maps(features, W0, b0, W1, b1, nodes2, neigh2, neigh1)
    kwargs = {}
    if _trace:
        import tempfile

        import ntff_shim  # noqa: F401  (registers the axon NTFF hook)

        kwargs = {"trace": True, "tmpdir": tempfile.mkdtemp(prefix="ntff_")}
    res = run_bass_kernel_spmd(nc, in_maps, list(range(N_CORES)), **kwargs)
    out = np.concatenate([res.results[c]["out"] for c in range(N_CORES)], axis=0)
    if _trace:
        _CACHE["last_result"] = res
    return out
